# revision 4
# baseline (speedup 1.0000x reference)
"""Trainium2 Bass kernel v2: multi-head attention (B=2, S=2048, E=1024, H=16).

Sharding: 8 cores = 2 batches x 4 head-groups; core c handles batch c//4,
heads [4*(c%4), 4*(c%4)+4).

Per-core program (all matmuls bf16, fp32 psum accumulate):
  - q/k projected directly into [d, s] layout (transposed matmuls); v into
    [s, c] layout with a ones column per head.
  - scores^T tiles [128 k, 512 q] on PE (sm_scale folded into Wq host-side);
    exp split across ACT (native Exp) and DVE/GPSIMD (Schraudolph:
    int16(A*s + B) bit-cast to bf16; the approximation's constant factor
    cancels in softmax normalization).
  - attn@v with attn^T chunks [128 k, 128 q] stationary and v [128 k, 65]
    moving -> psum [128 q, 65] (full 128-row contraction; col 64 = softmax
    denominator). reciprocal_approx_fast + per-partition scaled copy
    normalizes into o [s, c] bf16.
  - o transposed via PE identity-matmul, then output projection; partial
    [S, E] written bf16; host sums the 4 head-group partials per batch.
"""

import numpy as np
import ml_dtypes

import concourse.tile as tile
import concourse.mybir as mybir
from concourse import bacc
from concourse.bass_utils import run_bass_kernel_spmd

B, S, E, H, D = 2, 2048, 1024, 16, 64
NCORES = 8
GPB = NCORES // B      # head-group cores per batch = 4
HPC = H // GPB         # heads per core = 4
FPC = HPC * D          # feature cols per core = 256
SM = float(D) ** -0.5  # softmax scale (folded into Wq on host)

F32 = mybir.dt.float32
F8 = mybir.dt.float8e4
BF16 = mybir.dt.bfloat16
I16 = mybir.dt.int16

P = 128
NE = E // P            # 8 e-tiles (contraction chunks)
NST = S // P           # 16 s-tiles (key tiles)
NQ = 4                 # query chunks
QC = S // NQ           # 512
NJ = NST // 2          # 8 kt-pairs per (head, chunk)

import os

# Schraudolph exp in bf16: exp(s) ~= bitcast_bf16(int16(A16*s + B16));
# B16 = 16256 + c with c chosen so E[ln(approx/exp)] = 0 over the mantissa
# ripple -- matches the ACT-exp blocks so softmax block weights stay unbiased
A16 = 128.0 * 1.4426950408889634
B16 = float(os.environ.get("KV2_B16", "16249.5"))

# exp engine pattern per key tile kt (A=ACT native, D=DVE Schraudolph)
_PATS = {
    "AD": ["A", "A", "D", "A", "D", "A", "A", "D",
           "A", "D", "A", "A", "D", "A", "D", "A"],
    "AD8": ["A", "D", "A", "D", "A", "D", "A", "D",
            "A", "D", "A", "D", "A", "D", "A", "D"],
    "AD7": ["A", "D", "A", "D", "A", "D", "A", "A",
            "D", "A", "D", "A", "D", "A", "D", "A"],
    "ALLA": ["A"] * 16,
    "ALLD": ["D"] * 16,
    "AD6": ["A", "D", "A", "A", "D", "A", "D", "A",
            "A", "D", "A", "A", "D", "A", "D", "A"],
    "AD5": ["A", "D", "A", "A", "D", "A", "A", "D",
            "A", "A", "D", "A", "A", "D", "A", "A"],
    "AD4": ["A", "A", "D", "A", "A", "A", "D", "A",
            "A", "A", "D", "A", "A", "A", "D", "A"],
}
EXP_PAT = _PATS[os.environ.get("KV2_PAT", "AD7")]
PUMP = int(os.environ.get("KV2_PUMP", "1"))
SPOOL = int(os.environ.get("KV2_SPOOL", "4"))
MPOOL = int(os.environ.get("KV2_MPOOL", "2"))
GCDMA = os.environ.get("KV2_GCDMA", "0") == "1"
NORMSPREAD = os.environ.get("KV2_NORMSPREAD", "0") == "1"
LAG = int(os.environ.get("KV2_LAG", "8"))
FP8 = os.environ.get("KV2_FP8", "0") == "1"
APOOL = int(os.environ.get("KV2_APOOL", "10"))
XBATCH = os.environ.get("KV2_XBATCH", "0") == "1"
NORMLAG = int(os.environ.get("KV2_NORMLAG", "6"))


def _build():
    nc = bacc.Bacc("TRN2", target_bir_lowering=False, debug=False)

    xT_d = nc.dram_tensor("xT", [E, S], BF16, kind="ExternalInput")
    wq_d = nc.dram_tensor("wqT", [E, FPC], BF16, kind="ExternalInput")
    wk_d = nc.dram_tensor("wkT", [E, FPC], BF16, kind="ExternalInput")
    wv_d = nc.dram_tensor("wvT", [E, FPC], BF16, kind="ExternalInput")
    wo_d = nc.dram_tensor("woT", [FPC, E], BF16, kind="ExternalInput")
    id_d = nc.dram_tensor("ident", [P, P], BF16, kind="ExternalInput")
    out_d = nc.dram_tensor("out", [S, E], BF16, kind="ExternalOutput")
    DBG = os.environ.get("KV2_DEBUG", "0") == "1"
    if DBG:
        dbg_o = nc.dram_tensor("dbg_o", [P, FPC], BF16, kind="ExternalOutput")
        dbg_oT = nc.dram_tensor("dbg_oT", [P, 2, P], BF16, kind="ExternalOutput")
        dbg_rec = nc.dram_tensor("dbg_rec", [P, NQ], F32, kind="ExternalOutput")
        dbg_v = nc.dram_tensor("dbg_v", [P, HPC, D + 1], BF16, kind="ExternalOutput")
        dbg_k = nc.dram_tensor("dbg_k", [P, S], BF16, kind="ExternalOutput")
        dbg_q = nc.dram_tensor("dbg_q", [P, S], BF16, kind="ExternalOutput")
        dbg_at = nc.dram_tensor("dbg_at", [P, QC], BF16, kind="ExternalOutput")
        dbg_po = nc.dram_tensor("dbg_po", [P, NQ, D + 1], F32, kind="ExternalOutput")

    with tile.TileContext(nc) as tc:
        with (
            tc.tile_pool(name="wpool", bufs=1) as wpool,
            tc.tile_pool(name="xpool", bufs=1) as xpool,
            tc.tile_pool(name="qkpool", bufs=1) as qkpool,
            tc.tile_pool(name="vpool", bufs=1) as vpool,
            tc.tile_pool(name="apool", bufs=APOOL) as apool,
            tc.tile_pool(name="opool", bufs=1) as opool,
            tc.tile_pool(name="otpool", bufs=3) as otpool,
            tc.tile_pool(name="rpool", bufs=2) as rpool,
            tc.tile_pool(name="npool", bufs=2) as npool,
            tc.tile_pool(name="outpool", bufs=4) as outpool,
            tc.tile_pool(name="spool", bufs=SPOOL, space="PSUM") as spool,
            tc.tile_pool(name="oaccpool", bufs=2, space="PSUM") as oaccpool,
            tc.tile_pool(name="mpool", bufs=MPOOL, space="PSUM") as mpool,
        ):
            # ---- weights / constants -------------------------------------
            wk = wpool.tile([P, NE, FPC], BF16, name="wk")
            wq = wpool.tile([P, NE, FPC], BF16, name="wq")
            wv = wpool.tile([P, NE, FPC], BF16, name="wv")
            wo = wpool.tile([P, 2, E], BF16, name="wo")
            ident = wpool.tile([P, P], BF16, name="ident")

            wk_r = wk_d.ap().rearrange("(t p) f -> p t f", p=P)
            wq_r = wq_d.ap().rearrange("(t p) f -> p t f", p=P)
            # pair-0 halves of Wk/Wq first: they gate the first projections
            nc.sync.dma_start(out=wk[:, :, 0:P], in_=wk_r[:, :, 0:P])
            nc.sync.dma_start(out=wq[:, :, 0:P], in_=wq_r[:, :, 0:P])
            nc.sync.dma_start(
                out=wv, in_=wv_d.ap().rearrange("(t p) f -> p t f", p=P)
            )

            if XBATCH:
                xts_all = xpool.tile([P, NE, S], BF16, name="xt")
                xts = [xts_all[:, et, :] for et in range(NE)]
            else:
                xts = [
                    xpool.tile([P, S], BF16, name=f"xt{et}", tag=f"xt{et}")
                    for et in range(NE)
                ]
            xT_r = xT_d.ap().rearrange("(t p) s -> p t s", p=P)
            for et in range(NE):
                nc.sync.dma_start(out=xts[et][:, 0:QC], in_=xT_r[:, et, 0:QC])
            nc.sync.dma_start(out=wk[:, :, P:FPC], in_=wk_r[:, :, P:FPC])
            nc.sync.dma_start(out=wq[:, :, P:FPC], in_=wq_r[:, :, P:FPC])
            for cq in range(1, NQ):
                csl = slice(cq * QC, (cq + 1) * QC)
                if XBATCH:
                    nc.sync.dma_start(out=xts_all[:, :, csl], in_=xT_r[:, :, csl])
                else:
                    for et in range(NE):
                        nc.sync.dma_start(out=xts[et][:, csl], in_=xT_r[:, et, csl])
            nc.sync.dma_start(
                out=wo, in_=wo_d.ap().rearrange("(t p) g -> p t g", p=P)
            )
            nc.sync.dma_start(out=ident, in_=id_d.ap())

            # ---- persistent sbuf tensors ---------------------------------
            QKDT = F8 if FP8 else BF16
            kts = [qkpool.tile([P, S], QKDT, name=f"kt{p}", tag=f"kt{p}") for p in range(2)]
            qts = [qkpool.tile([P, S], QKDT, name=f"qt{p}", tag=f"qt{p}") for p in range(2)]
            if FP8:
                kf8 = [qkpool.tile([P, 2, S], F8, name=f"kf8{p}", tag=f"kf8{p}") for p in range(2)]
                qf8 = [qkpool.tile([P, 2, S], F8, name=f"qf8{p}", tag=f"qf8{p}") for p in range(2)]

            def shuffle_f8(stage, dst, csl):
                """stage [128, S] fp8 -> dst [128, 2, S]: head tiles at base
                partitions {0,64}; contraction d = sub*32 + p."""
                st_r = stage.rearrange("(b u p) s -> b u p s", b=2, u=2)
                dst_r = dst.rearrange("(b u p) t s -> b u p t s", b=2, u=2)
                for sub in range(2):
                    nc.sync.dma_start(
                        out=dst_r[:, 0, :, sub, csl], in_=st_r[:, sub, :, csl]
                    )
            v_sb = [
                vpool.tile([P, HPC, D + 1], BF16, name=f"v{st}", tag=f"v{st}")
                for st in range(NST)
            ]
            # ones columns for softmax denominators (idle Pool engine, early)
            for st in range(NST):
                nc.gpsimd.memset(v_sb[st][:, :, D : D + 1], 1.0)
            o_sb = [
                opool.tile([P, FPC], BF16, name=f"o{st}", tag=f"o{st % 8}", bufs=2)
                for st in range(NST)
            ]

            # ---- filler machinery (PE backfill units) --------------------
            from collections import deque

            fillers = deque()
            ready = set()

            def pump(n):
                for _ in range(n):
                    while fillers:
                        try:
                            next(fillers[0])
                            break
                        except StopIteration:
                            fillers.popleft()
                    else:
                        return

            def pump_until(key):
                # force-drain fillers until the unit producing `key` has been
                # fully EMITTED (emission order defines Tile dependencies)
                while key not in ready:
                    assert fillers, f"no filler can produce {key}"
                    try:
                        next(fillers[0])
                    except StopIteration:
                        fillers.popleft()

            def kq_proj(w_tile, dst, p, cq, copy_eng=None):
                """dst[p][:, cq chunk] = (x @ W_pslice^T)^T  in [d, s] layout."""
                ps = mpool.tile([P, QC], F32, name="ps_kq", tag="m")
                for et in range(NE):
                    nc.tensor.matmul(
                        ps,
                        w_tile[:, et, p * P : (p + 1) * P],
                        xts[et][:, cq * QC : (cq + 1) * QC],
                        start=(et == 0),
                        stop=(et == NE - 1),
                    )
                    yield
                csl = slice(cq * QC, (cq + 1) * QC)
                dslice = dst[p][:, csl]
                if copy_eng == "A":
                    nc.scalar.activation(
                        out=dslice, in_=ps,
                        func=mybir.ActivationFunctionType.Copy,
                    )
                else:
                    nc.vector.tensor_copy(dslice, ps)
                if FP8 and dst in (kts, qts):
                    shuffle_f8(dst[p], (kf8 if dst is kts else qf8)[p], csl)
                yield

            def v_proj(st):
                """v_sb[st][:, h, 0:D] = x s-tile @ Wv^T (all 4 heads)."""
                ps = mpool.tile([P, FPC], F32, name="ps_v", tag="m")
                for et in range(NE):
                    nc.tensor.matmul(
                        ps,
                        xts[et][:, st * P : (st + 1) * P],
                        wv[:, et, :],
                        start=(et == 0),
                        stop=(et == NE - 1),
                    )
                    yield
                nc.scalar.activation(
                    out=v_sb[st][:, :, 0:D],
                    in_=ps.rearrange("p (h d) -> p h d", d=D),
                    func=mybir.ActivationFunctionType.Copy,
                )
                yield

            def run_now(gen):
                for _ in gen:
                    pass

            # ---- upfront: k/q chunk 0 (pair 0), v tiles 0-1; rest filler --
            run_now(kq_proj(wk, kts, 0, 0))
            run_now(kq_proj(wq, qts, 0, 0, copy_eng="A"))
            run_now(v_proj(0))
            run_now(v_proj(1))
            ready.update({"k0c0", "q0c0", "v0", "v1"})

            def filler_projs():
                yield from v_proj(2)
                ready.add("v2")
                yield from v_proj(3)
                ready.add("v3")
                yield from kq_proj(wk, kts, 0, 1)
                ready.add("k0c1")
                for st in (4, 5, 6, 7):
                    yield from v_proj(st)
                    ready.add(f"v{st}")
                yield from kq_proj(wk, kts, 0, 2)
                ready.add("k0c2")
                for st in (8, 9, 10, 11):
                    yield from v_proj(st)
                    ready.add(f"v{st}")
                yield from kq_proj(wk, kts, 0, 3)
                ready.add("k0c3")
                for st in range(12, NST):
                    yield from v_proj(st)
                    ready.add(f"v{st}")
                yield from kq_proj(wq, qts, 1, 0, copy_eng="A")
                ready.add("q1c0")
                for cq in range(NQ):
                    yield from kq_proj(wk, kts, 1, cq)
                    ready.add(f"k1c{cq}")
                for cq in range(1, NQ):
                    yield from kq_proj(wq, qts, 0, cq, copy_eng="A")
                    ready.add(f"q0c{cq}")
                    yield from kq_proj(wq, qts, 1, cq)
                    ready.add(f"q1c{cq}")

            fillers.append(filler_projs())

            # ---- attention core ------------------------------------------
            rec_keep = []
            dbg_attn_keep = []

            def attn_all():
                """All (cq, h) attention as a single software-pipelined
                (cq, h, kt) stream: attn@v lags LAG steps behind the
                score/exp stream, crossing head AND chunk boundaries."""
                ps_os = {}
                attn_ts = {}
                pend = []
                norm_pend = []
                step = [0]

                def flush_norms(min_age):
                    while norm_pend and step[0] - norm_pend[0][0] >= min_age:
                        _, ncq, nh, nps = norm_pend.pop(0)
                        normalize(nh, ncq, nps)
                        if nh == HPC - 1:
                            fillers.append(finish(ncq))

                def attn_v(cq, h, kt):
                    at = attn_ts.pop((cq, h, kt))
                    ps_o = ps_os[(cq, h)]
                    pump_until(f"v{kt}")
                    for qt in range(NQ):
                        # start=True clears has_written for the WHOLE bank:
                        # only the first matmul may carry it; qt>0 at kt==0
                        # rely on cleared bits -> overwrite semantics.
                        nc.tensor.matmul(
                            ps_o[:, qt, :],
                            at[:, qt * P : (qt + 1) * P],
                            v_sb[kt][:, h, :],
                            start=(kt == 0 and qt == 0),
                            stop=(kt == NST - 1 and qt == NQ - 1),
                            skip_group_check=(kt == 0 and qt > 0),
                        )
                    if kt == NST - 1:
                        norm_pend.append((step[0], cq, h, ps_os.pop((cq, h))))

                for cq in range(NQ):
                    for h in range(HPC):
                        p, sub = h // 2, h % 2
                        lo = sub * D
                        csl = slice(cq * QC, (cq + 1) * QC)
                        ps_os[(cq, h)] = oaccpool.tile(
                            [P, NQ, D + 1], F32, name="ps_o", tag="oacc"
                        )
                        pump_until(f"q{p}c{cq}")
                        for kt in range(NST):
                            pump_until(f"k{p}c{kt // NQ}")
                            sps = spool.tile([P, QC], F32, name="sps", tag="sps")
                            if FP8:
                                nc.tensor.matmul(
                                    sps,
                                    kf8[p][lo : lo + 32, :, kt * P : (kt + 1) * P],
                                    qf8[p][lo : lo + 32, :, csl],
                                    start=True,
                                    stop=True,
                                    perf_mode=mybir.MatmulPerfMode.DoubleRow,
                                )
                            else:
                                nc.tensor.matmul(
                                    sps,
                                    kts[p][lo : lo + D, kt * P : (kt + 1) * P],
                                    qts[p][lo : lo + D, csl],
                                    start=True,
                                    stop=True,
                                )
                            pump(PUMP)
                            attn_t = apool.tile(
                                [P, QC], BF16, name="attn", tag="attn"
                            )
                            attn_ts[(cq, h, kt)] = attn_t
                            if EXP_PAT[kt] == "A":
                                nc.scalar.activation(
                                    out=attn_t,
                                    in_=sps,
                                    func=mybir.ActivationFunctionType.Exp,
                                )
                            else:
                                nc.vector.tensor_scalar(
                                    attn_t.bitcast(I16),
                                    sps,
                                    A16,
                                    B16,
                                    mybir.AluOpType.mult,
                                    mybir.AluOpType.add,
                                )
                            pend.append((cq, h, kt))
                            step[0] += 1
                            if len(pend) > LAG:
                                attn_v(*pend.pop(0))
                                flush_norms(NORMLAG)
                                pump(PUMP)
                while pend:
                    attn_v(*pend.pop(0))
                flush_norms(0)

            def normalize(h, cq, ps_o):
                """Evacuate ps_o once (ACT/DVE), then denominator handling and
                the scaled copies run on the otherwise-idle GPSIMD engine so
                the exp engines stay dense."""
                po_sb = npool.tile([P, NQ, D + 1], F32, name="po_sb", tag="po_sb")
                if h % 2 == 0:
                    nc.scalar.activation(
                        out=po_sb, in_=ps_o,
                        func=mybir.ActivationFunctionType.Copy,
                    )
                else:
                    nc.vector.tensor_copy(po_sb, ps_o)
                den = rpool.tile([P, NQ], F32, name="den", tag="den")
                nc.gpsimd.tensor_copy(den, po_sb[:, :, D])
                rec = rpool.tile([P, NQ], F32, name="rec", tag="rec")
                rec_keep.append(rec)
                nc.vector.reciprocal(rec, den)
                for qt in range(NQ):
                    st = cq * NQ + qt
                    nc.gpsimd.tensor_scalar(
                        o_sb[st][:, h * D : (h + 1) * D],
                        po_sb[:, qt, 0:D],
                        rec[:, qt : qt + 1],
                        None,
                        mybir.AluOpType.mult,
                    )

            def finish(cq):
                """Transpose + output projection for the 4 s-tiles of cq.
                The last chunk runs post-attention: use the freed score psum
                pool for deeper pipelining and the idle ACT/DVE for copies."""
                last = cq == NQ - 1
                for qt in range(NQ):
                    st = cq * NQ + qt
                    oT = otpool.tile([P, 2, P], BF16, name="oT", tag="oT")
                    for ct in range(2):
                        tp = mpool.tile([P, P], BF16, name="tp", tag="m")
                        nc.tensor.transpose(
                            tp, o_sb[st][:, ct * P : (ct + 1) * P], ident
                        )
                        yield
                        nc.vector.tensor_copy(oT[:, ct, :], tp)
                        yield
                    if DBG and st == 0:
                        nc.sync.dma_start(out=dbg_oT.ap(), in_=oT)
                    out_sb = outpool.tile([P, E], BF16, name="out_sb", tag="out_sb")
                    for gc in range(2):
                        if last:
                            po = spool.tile([P, QC], F32, name="sps", tag="sps")
                        else:
                            po = mpool.tile([P, QC], F32, name="po", tag="m")
                        for ct in range(2):
                            nc.tensor.matmul(
                                po,
                                oT[:, ct, :],
                                wo[:, ct, gc * QC : (gc + 1) * QC],
                                start=(ct == 0),
                                stop=(ct == 1),
                            )
                            yield
                        gsl = slice(gc * QC, (gc + 1) * QC)
                        if gc == 0:
                            nc.scalar.activation(
                                out=out_sb[:, gsl], in_=po,
                                func=mybir.ActivationFunctionType.Copy,
                            )
                        else:
                            nc.vector.tensor_copy(out_sb[:, gsl], po)
                        yield
                        if GCDMA:
                            nc.sync.dma_start(
                                out=out_d.ap()[st * P : (st + 1) * P, gsl],
                                in_=out_sb[:, gsl],
                            )
                    if not GCDMA:
                        nc.sync.dma_start(
                            out=out_d.ap()[st * P : (st + 1) * P, :], in_=out_sb
                        )

            attn_all()
            while fillers:
                pump(64)

    nc.compile()
    return nc


_NC_CACHE = None


def _get_nc():
    global _NC_CACHE
    if _NC_CACHE is None:
        _NC_CACHE = _build()
    return _NC_CACHE


def _bf16(a):
    return np.ascontiguousarray(a.astype(ml_dtypes.bfloat16))


def make_in_maps(x, Wq, Wk, Wv, Wo):
    in_maps = []
    xTs = [_bf16(x[b].T) for b in range(B)]
    ident = np.eye(P, dtype=ml_dtypes.bfloat16)
    for c in range(NCORES):
        b, hg = c // GPB, c % GPB
        fsl = slice(hg * FPC, (hg + 1) * FPC)
        in_maps.append({
            "xT": xTs[b],
            "wqT": _bf16(Wq[fsl, :].T * SM),
            "wkT": _bf16(Wk[fsl, :].T),
            "wvT": _bf16(Wv[fsl, :].T),
            "woT": _bf16(Wo[:, fsl].T),
            "ident": ident,
        })
    return in_maps


def kernel(x, Wq, bq, Wk, bk, Wv, bv, Wo, bo):
    x = np.asarray(x, dtype=np.float32)
    Wq, Wk, Wv, Wo = (np.asarray(a, dtype=np.float32) for a in (Wq, Wk, Wv, Wo))
    bq, bk, bv, bo = (np.asarray(a, dtype=np.float32) for a in (bq, bk, bv, bo))
    if np.any(bq) or np.any(bk) or np.any(bv):
        raise NotImplementedError("nonzero projection biases not supported")

    nc = _get_nc()
    in_maps = make_in_maps(x, Wq, Wk, Wv, Wo)
    res = run_bass_kernel_spmd(nc, in_maps, core_ids=list(range(NCORES)))
    out = np.empty((B, S, E), dtype=np.float32)
    for b in range(B):
        acc = res.results[b * GPB]["out"].astype(np.float32)
        for hg in range(1, GPB):
            acc = acc + res.results[b * GPB + hg]["out"].astype(np.float32)
        out[b] = acc
    out += bo[None, None, :]
    return out


# revision 6
# speedup vs baseline: 1.0282x; 1.0282x over previous
"""Trainium2 Bass kernel v2: multi-head attention (B=2, S=2048, E=1024, H=16).

Sharding: 8 cores = 2 batches x 4 head-groups; core c handles batch c//4,
heads [4*(c%4), 4*(c%4)+4).

Per-core program (all matmuls bf16, fp32 psum accumulate):
  - q/k projected directly into [d, s] layout (transposed matmuls); v into
    [s, c] layout with a ones column per head.
  - scores^T tiles [128 k, 512 q] on PE (sm_scale folded into Wq host-side);
    exp split across ACT (native Exp) and DVE/GPSIMD (Schraudolph:
    int16(A*s + B) bit-cast to bf16; the approximation's constant factor
    cancels in softmax normalization).
  - attn@v with attn^T chunks [128 k, 128 q] stationary and v [128 k, 65]
    moving -> psum [128 q, 65] (full 128-row contraction; col 64 = softmax
    denominator). reciprocal_approx_fast + per-partition scaled copy
    normalizes into o [s, c] bf16.
  - o transposed via PE identity-matmul, then output projection; partial
    [S, E] written bf16; host sums the 4 head-group partials per batch.
"""

import numpy as np
import ml_dtypes

import concourse.tile as tile
import concourse.mybir as mybir
from concourse import bacc
from concourse.bass_utils import run_bass_kernel_spmd

B, S, E, H, D = 2, 2048, 1024, 16, 64
NCORES = 8
GPB = NCORES // B      # head-group cores per batch = 4
HPC = H // GPB         # heads per core = 4
FPC = HPC * D          # feature cols per core = 256
SM = float(D) ** -0.5  # softmax scale (folded into Wq on host)

F32 = mybir.dt.float32
F8 = mybir.dt.float8e4
BF16 = mybir.dt.bfloat16
I16 = mybir.dt.int16

P = 128
NE = E // P            # 8 e-tiles (contraction chunks)
NST = S // P           # 16 s-tiles (key tiles)
NQ = 4                 # query chunks
QC = S // NQ           # 512
NJ = NST // 2          # 8 kt-pairs per (head, chunk)

import os

# Schraudolph exp in bf16: exp(s) ~= bitcast_bf16(int16(A16*s + B16));
# B16 = 16256 + c with c chosen so E[ln(approx/exp)] = 0 over the mantissa
# ripple -- matches the ACT-exp blocks so softmax block weights stay unbiased
A16 = 128.0 * 1.4426950408889634
B16 = float(os.environ.get("KV2_B16", "16249.5"))

# exp engine pattern per key tile kt (A=ACT native, D=DVE Schraudolph)
_PATS = {
    "AD": ["A", "A", "D", "A", "D", "A", "A", "D",
           "A", "D", "A", "A", "D", "A", "D", "A"],
    "AD8": ["A", "D", "A", "D", "A", "D", "A", "D",
            "A", "D", "A", "D", "A", "D", "A", "D"],
    "AD7": ["A", "D", "A", "D", "A", "D", "A", "A",
            "D", "A", "D", "A", "D", "A", "D", "A"],
    "ALLA": ["A"] * 16,
    "ALLD": ["D"] * 16,
    "AD6": ["A", "D", "A", "A", "D", "A", "D", "A",
            "A", "D", "A", "A", "D", "A", "D", "A"],
    "AD5": ["A", "D", "A", "A", "D", "A", "A", "D",
            "A", "A", "D", "A", "A", "D", "A", "A"],
    "AD4": ["A", "A", "D", "A", "A", "A", "D", "A",
            "A", "A", "D", "A", "A", "A", "D", "A"],
}
EXP_PAT = _PATS[os.environ.get("KV2_PAT", "AD7")]
PUMP = int(os.environ.get("KV2_PUMP", "1"))
SPOOL = int(os.environ.get("KV2_SPOOL", "4"))
MPOOL = int(os.environ.get("KV2_MPOOL", "2"))
GCDMA = os.environ.get("KV2_GCDMA", "0") == "1"
NORMSPREAD = os.environ.get("KV2_NORMSPREAD", "0") == "1"
LAG = int(os.environ.get("KV2_LAG", "12"))
FP8 = os.environ.get("KV2_FP8", "0") == "1"
APOOL = int(os.environ.get("KV2_APOOL", "14"))
XBATCH = os.environ.get("KV2_XBATCH", "0") == "1"
NORMLAG = int(os.environ.get("KV2_NORMLAG", "4"))
WEXP = os.environ.get("KV2_WEXP", "0") == "1"
GPAT = os.environ.get("KV2_GPAT", "ADADADAAADADADAD")
PROJ8 = os.environ.get("KV2_PROJ8", "0") == "1"
V8 = os.environ.get("KV2_V8", "0") == "1"


def _build():
    nc = bacc.Bacc("TRN2", target_bir_lowering=False, debug=False)

    xT_d = nc.dram_tensor("xT", [E, S], BF16, kind="ExternalInput")
    wq_d = nc.dram_tensor("wqT", [E, FPC], BF16, kind="ExternalInput")
    wk_d = nc.dram_tensor("wkT", [E, FPC], BF16, kind="ExternalInput")
    wv_d = nc.dram_tensor("wvT", [E, FPC], BF16, kind="ExternalInput")
    wo_d = nc.dram_tensor("woT", [FPC, E], BF16, kind="ExternalInput")
    id_d = nc.dram_tensor("ident", [P, P], BF16, kind="ExternalInput")
    if PROJ8 or V8:
        x8_d = nc.dram_tensor("x8", [E, S], F8, kind="ExternalInput")
    if PROJ8:
        wq8_d = nc.dram_tensor("wq8", [E, FPC], F8, kind="ExternalInput")
        wk8_d = nc.dram_tensor("wk8", [E, FPC], F8, kind="ExternalInput")
    if V8:
        wv8_d = nc.dram_tensor("wv8", [E, FPC], F8, kind="ExternalInput")
    out_d = nc.dram_tensor("out", [S, E], BF16, kind="ExternalOutput")
    DBG = os.environ.get("KV2_DEBUG", "0") == "1"
    if DBG:
        dbg_o = nc.dram_tensor("dbg_o", [P, FPC], BF16, kind="ExternalOutput")
        dbg_oT = nc.dram_tensor("dbg_oT", [P, 2, P], BF16, kind="ExternalOutput")
        dbg_rec = nc.dram_tensor("dbg_rec", [P, NQ], F32, kind="ExternalOutput")
        dbg_v = nc.dram_tensor("dbg_v", [P, HPC, D + 1], BF16, kind="ExternalOutput")
        dbg_k = nc.dram_tensor("dbg_k", [P, S], BF16, kind="ExternalOutput")
        dbg_q = nc.dram_tensor("dbg_q", [P, S], BF16, kind="ExternalOutput")
        dbg_at = nc.dram_tensor("dbg_at", [P, QC], BF16, kind="ExternalOutput")
        dbg_po = nc.dram_tensor("dbg_po", [P, NQ, D + 1], F32, kind="ExternalOutput")

    with tile.TileContext(nc) as tc:
        with (
            tc.tile_pool(name="wpool", bufs=1) as wpool,
            tc.tile_pool(name="xpool", bufs=1) as xpool,
            tc.tile_pool(name="qkpool", bufs=1) as qkpool,
            tc.tile_pool(name="vpool", bufs=1) as vpool,
            tc.tile_pool(name="apool", bufs=APOOL) as apool,
            tc.tile_pool(name="opool", bufs=1) as opool,
            tc.tile_pool(name="otpool", bufs=3) as otpool,
            tc.tile_pool(name="rpool", bufs=2) as rpool,
            tc.tile_pool(name="npool", bufs=2) as npool,
            tc.tile_pool(name="outpool", bufs=4) as outpool,
            tc.tile_pool(name="spool", bufs=SPOOL, space="PSUM") as spool,
            tc.tile_pool(name="oaccpool", bufs=2, space="PSUM") as oaccpool,
            tc.tile_pool(name="mpool", bufs=MPOOL, space="PSUM") as mpool,
        ):
            # ---- weights / constants -------------------------------------
            wk = wpool.tile([P, NE, FPC], BF16, name="wk")
            wq = wpool.tile([P, NE, FPC], BF16, name="wq")
            wv = wpool.tile([P, NE, FPC], BF16, name="wv")
            wo = wpool.tile([P, 2, E], BF16, name="wo")
            ident = wpool.tile([P, P], BF16, name="ident")

            wk_r = wk_d.ap().rearrange("(t p) f -> p t f", p=P)
            wq_r = wq_d.ap().rearrange("(t p) f -> p t f", p=P)
            if PROJ8:
                # e = pr*256 + sub*128 + p: DoubleRow pairs of e-chunks
                wk8 = wpool.tile([P, NE // 2, 2, FPC], F8, name="wk8")
                wq8 = wpool.tile([P, NE // 2, 2, FPC], F8, name="wq8")
                x8 = xpool.tile([P, NE // 2, 2, S], F8, name="x8")
                wk8_r = wk8_d.ap().rearrange("(r u p) f -> p r u f", p=P, u=2)
                wq8_r = wq8_d.ap().rearrange("(r u p) f -> p r u f", p=P, u=2)
                x8_r = x8_d.ap().rearrange("(r u p) s -> p r u s", p=P, u=2)
                nc.sync.dma_start(out=wk8, in_=wk8_r)
                nc.sync.dma_start(out=wq8, in_=wq8_r)
                nc.sync.dma_start(out=x8[:, :, :, 0:QC], in_=x8_r[:, :, :, 0:QC])
            else:
                # pair-0 halves of Wk/Wq first: they gate the first projections
                nc.sync.dma_start(out=wk[:, :, 0:P], in_=wk_r[:, :, 0:P])
                nc.sync.dma_start(out=wq[:, :, 0:P], in_=wq_r[:, :, 0:P])

            if XBATCH:
                xts_all = xpool.tile([P, NE, S], BF16, name="xt")
                xts = [xts_all[:, et, :] for et in range(NE)]
            else:
                xts = [
                    xpool.tile([P, S], BF16, name=f"xt{et}", tag=f"xt{et}")
                    for et in range(NE)
                ]
            if V8:
                wv8 = wpool.tile([P, NE // 2, 2, FPC], F8, name="wv8")
                x8v = xpool.tile([P, NE // 2, 2, S], F8, name="x8v")
                x8v_r = x8_d.ap().rearrange("(r u p) s -> p r u s", p=P, u=2)
                nc.sync.dma_start(
                    out=wv8, in_=wv8_d.ap().rearrange("(r u p) f -> p r u f", p=P, u=2)
                )
            xT_r = xT_d.ap().rearrange("(t p) s -> p t s", p=P)
            for et in range(NE):
                nc.sync.dma_start(out=xts[et][:, 0:QC], in_=xT_r[:, et, 0:QC])
            if V8:
                nc.sync.dma_start(out=x8v[:, :, :, 0:QC], in_=x8v_r[:, :, :, 0:QC])
            nc.sync.dma_start(
                out=wv, in_=wv_d.ap().rearrange("(t p) f -> p t f", p=P)
            )
            if not PROJ8:
                nc.sync.dma_start(out=wk[:, :, P:FPC], in_=wk_r[:, :, P:FPC])
                nc.sync.dma_start(out=wq[:, :, P:FPC], in_=wq_r[:, :, P:FPC])
            for cq in range(1, NQ):
                csl = slice(cq * QC, (cq + 1) * QC)
                if PROJ8:
                    nc.sync.dma_start(out=x8[:, :, :, csl], in_=x8_r[:, :, :, csl])
                for et in range(NE):
                    nc.sync.dma_start(out=xts[et][:, csl], in_=xT_r[:, et, csl])
                if V8:
                    nc.sync.dma_start(out=x8v[:, :, :, csl], in_=x8v_r[:, :, :, csl])
            nc.sync.dma_start(
                out=wo, in_=wo_d.ap().rearrange("(t p) g -> p t g", p=P)
            )
            nc.sync.dma_start(out=ident, in_=id_d.ap())

            # ---- persistent sbuf tensors ---------------------------------
            QKDT = F8 if FP8 else BF16
            kts = [qkpool.tile([P, S], QKDT, name=f"kt{p}", tag=f"kt{p}") for p in range(2)]
            qts = [qkpool.tile([P, S], QKDT, name=f"qt{p}", tag=f"qt{p}") for p in range(2)]
            if FP8:
                kf8 = [qkpool.tile([P, 2, S], F8, name=f"kf8{p}", tag=f"kf8{p}") for p in range(2)]
                qf8 = [qkpool.tile([P, 2, S], F8, name=f"qf8{p}", tag=f"qf8{p}") for p in range(2)]

            def shuffle_f8(stage, dst, csl):
                """stage [128, S] fp8 -> dst [128, 2, S]: head tiles at base
                partitions {0,64}; contraction d = sub*32 + p."""
                st_r = stage.rearrange("(b u p) s -> b u p s", b=2, u=2)
                dst_r = dst.rearrange("(b u p) t s -> b u p t s", b=2, u=2)
                for sub in range(2):
                    nc.sync.dma_start(
                        out=dst_r[:, 0, :, sub, csl], in_=st_r[:, sub, :, csl]
                    )
            v_sb = [
                vpool.tile([P, HPC, D + 1], BF16, name=f"v{st}", tag=f"v{st}")
                for st in range(NST)
            ]
            # ones columns for softmax denominators (idle Pool engine, early)
            for st in range(NST):
                nc.gpsimd.memset(v_sb[st][:, :, D : D + 1], 1.0)
            o_sb = [
                opool.tile([P, FPC], BF16, name=f"o{st}", tag=f"o{st % 8}", bufs=2)
                for st in range(NST)
            ]

            # ---- filler machinery (PE backfill units) --------------------
            from collections import deque

            fillers = deque()
            ready = set()

            def pump(n):
                for _ in range(n):
                    while fillers:
                        try:
                            next(fillers[0])
                            break
                        except StopIteration:
                            fillers.popleft()
                    else:
                        return

            def pump_until(key):
                # force-drain fillers until the unit producing `key` has been
                # fully EMITTED (emission order defines Tile dependencies)
                while key not in ready:
                    assert fillers, f"no filler can produce {key}"
                    try:
                        next(fillers[0])
                    except StopIteration:
                        fillers.popleft()

            def kq_proj(w_tile, dst, p, cq, copy_eng=None):
                """dst[p][:, cq chunk] = (x @ W_pslice^T)^T  in [d, s] layout."""
                ps = mpool.tile([P, QC], F32, name="ps_kq", tag="m")
                if PROJ8:
                    w8t = wk8 if dst is kts else wq8
                    for pr in range(NE // 2):
                        nc.tensor.matmul(
                            ps,
                            w8t[:, pr, :, p * P : (p + 1) * P],
                            x8[:, pr, :, cq * QC : (cq + 1) * QC],
                            start=(pr == 0),
                            stop=(pr == NE // 2 - 1),
                            perf_mode=mybir.MatmulPerfMode.DoubleRow,
                        )
                        yield
                else:
                    for et in range(NE):
                        nc.tensor.matmul(
                            ps,
                            w_tile[:, et, p * P : (p + 1) * P],
                            xts[et][:, cq * QC : (cq + 1) * QC],
                            start=(et == 0),
                            stop=(et == NE - 1),
                        )
                        yield
                csl = slice(cq * QC, (cq + 1) * QC)
                dslice = dst[p][:, csl]
                if copy_eng == "A":
                    nc.scalar.activation(
                        out=dslice, in_=ps,
                        func=mybir.ActivationFunctionType.Copy,
                    )
                else:
                    nc.vector.tensor_copy(dslice, ps)
                if FP8 and dst in (kts, qts):
                    shuffle_f8(dst[p], (kf8 if dst is kts else qf8)[p], csl)
                yield

            def v_proj(st):
                """v_sb[st][:, h, 0:D] = x s-tile @ Wv^T (all 4 heads).
                V8: fp8 DoubleRow with x16-lifted operands -> v scaled by 256;
                compensated in the softmax denominator scaling."""
                ps = mpool.tile([P, FPC], F32, name="ps_v", tag="m")
                if V8:
                    for pr in range(NE // 2):
                        nc.tensor.matmul(
                            ps,
                            x8v[:, pr, :, st * P : (st + 1) * P],
                            wv8[:, pr, :, :],
                            start=(pr == 0),
                            stop=(pr == NE // 2 - 1),
                            perf_mode=mybir.MatmulPerfMode.DoubleRow,
                        )
                        yield
                else:
                    for et in range(NE):
                        nc.tensor.matmul(
                            ps,
                            xts[et][:, st * P : (st + 1) * P],
                            wv[:, et, :],
                            start=(et == 0),
                            stop=(et == NE - 1),
                        )
                        yield
                nc.scalar.activation(
                    out=v_sb[st][:, :, 0:D],
                    in_=ps.rearrange("p (h d) -> p h d", d=D),
                    func=mybir.ActivationFunctionType.Copy,
                )
                yield

            def run_now(gen):
                for _ in gen:
                    pass

            # ---- upfront: k/q chunk 0 (pair 0), v tiles 0-1; rest filler --
            run_now(kq_proj(wk, kts, 0, 0))
            run_now(kq_proj(wq, qts, 0, 0, copy_eng="A"))
            ready.update({"k0c0", "q0c0"})

            def filler_projs():
                for st in (0, 1, 2, 3):
                    yield from v_proj(st)
                    ready.add(f"v{st}")
                yield from kq_proj(wk, kts, 0, 1)
                ready.add("k0c1")
                for st in (4, 5, 6, 7):
                    yield from v_proj(st)
                    ready.add(f"v{st}")
                yield from kq_proj(wk, kts, 0, 2)
                ready.add("k0c2")
                for st in (8, 9, 10, 11):
                    yield from v_proj(st)
                    ready.add(f"v{st}")
                yield from kq_proj(wk, kts, 0, 3)
                ready.add("k0c3")
                for st in range(12, NST):
                    yield from v_proj(st)
                    ready.add(f"v{st}")
                yield from kq_proj(wq, qts, 1, 0, copy_eng="A")
                ready.add("q1c0")
                for cq in range(NQ):
                    yield from kq_proj(wk, kts, 1, cq)
                    ready.add(f"k1c{cq}")
                for cq in range(1, NQ):
                    yield from kq_proj(wq, qts, 0, cq, copy_eng="A")
                    ready.add(f"q0c{cq}")
                    yield from kq_proj(wq, qts, 1, cq)
                    ready.add(f"q1c{cq}")

            fillers.append(filler_projs())

            # ---- attention core ------------------------------------------
            rec_keep = []
            dbg_attn_keep = []

            def attn_all():
                """All (cq, h) attention as a single software-pipelined
                (cq, h, kt) stream: attn@v lags LAG steps behind the
                score/exp stream, crossing head AND chunk boundaries."""
                ps_os = {}
                attn_ts = {}
                pend = []
                norm_pend = []
                step = [0]

                def flush_norms(min_age):
                    while norm_pend and step[0] - norm_pend[0][0] >= min_age:
                        _, ncq, nh, nps = norm_pend.pop(0)
                        normalize(nh, ncq, nps)
                        if nh == HPC - 1:
                            fillers.append(finish(ncq))

                def attn_v(cq, h, kt):
                    at = attn_ts.pop((cq, h, kt))
                    ps_o = ps_os[(cq, h)]
                    pump_until(f"v{kt}")
                    for qt in range(NQ):
                        # start=True clears has_written for the WHOLE bank:
                        # only the first matmul may carry it; qt>0 at kt==0
                        # rely on cleared bits -> overwrite semantics.
                        nc.tensor.matmul(
                            ps_o[:, qt, :],
                            at[:, qt * P : (qt + 1) * P],
                            v_sb[kt][:, h, :],
                            start=(kt == 0 and qt == 0),
                            stop=(kt == NST - 1 and qt == NQ - 1),
                            skip_group_check=(kt == 0 and qt > 0),
                        )
                    if kt == NST - 1:
                        norm_pend.append((step[0], cq, h, ps_os.pop((cq, h))))

                for cq in range(NQ):
                    for h in range(HPC):
                        p, sub = h // 2, h % 2
                        lo = sub * D
                        csl = slice(cq * QC, (cq + 1) * QC)
                        ps_os[(cq, h)] = oaccpool.tile(
                            [P, NQ, D + 1], F32, name="ps_o", tag="oacc"
                        )
                        pump_until(f"q{p}c{cq}")
                        sps2 = None
                        for kt in range(NST):
                            pump_until(f"k{p}c{kt // NQ}")
                            if WEXP:
                                if kt % 2 == 0:
                                    sps2 = spool.tile(
                                        [P, 2, QC], F32, name="sps", tag="sps"
                                    )
                                tgt = sps2[:, kt % 2, :]
                            else:
                                tgt = spool.tile([P, QC], F32, name="sps", tag="sps")
                            if FP8:
                                nc.tensor.matmul(
                                    tgt,
                                    kf8[p][lo : lo + 32, :, kt * P : (kt + 1) * P],
                                    qf8[p][lo : lo + 32, :, csl],
                                    start=True,
                                    stop=True,
                                    perf_mode=mybir.MatmulPerfMode.DoubleRow,
                                )
                            else:
                                nc.tensor.matmul(
                                    tgt,
                                    kts[p][lo : lo + D, kt * P : (kt + 1) * P],
                                    qts[p][lo : lo + D, csl],
                                    start=True,
                                    stop=True,
                                )
                            pump(PUMP)
                            if WEXP:
                                if kt % 2 == 1:
                                    attn_t = apool.tile(
                                        [P, 2, QC], BF16, name="attn", tag="attn"
                                    )
                                    attn_ts[(cq, h, kt - 1)] = attn_t[:, 0, :]
                                    attn_ts[(cq, h, kt)] = attn_t[:, 1, :]
                                    g = (h % 2) * 8 + kt // 2
                                    if GPAT[g] == "A":
                                        nc.scalar.activation(
                                            out=attn_t,
                                            in_=sps2,
                                            func=mybir.ActivationFunctionType.Exp,
                                        )
                                    else:
                                        nc.vector.tensor_scalar(
                                            attn_t.bitcast(I16),
                                            sps2,
                                            A16,
                                            B16,
                                            mybir.AluOpType.mult,
                                            mybir.AluOpType.add,
                                        )
                            else:
                                attn_t = apool.tile(
                                    [P, QC], BF16, name="attn", tag="attn"
                                )
                                attn_ts[(cq, h, kt)] = attn_t
                                esc = SM / 256.0 if PROJ8 else 1.0
                                if EXP_PAT[kt] == "A":
                                    nc.scalar.activation(
                                        out=attn_t,
                                        in_=tgt,
                                        func=mybir.ActivationFunctionType.Exp,
                                        scale=esc,
                                    )
                                else:
                                    nc.vector.tensor_scalar(
                                        attn_t.bitcast(I16),
                                        tgt,
                                        A16 * esc,
                                        B16,
                                        mybir.AluOpType.mult,
                                        mybir.AluOpType.add,
                                    )
                            pend.append((cq, h, kt))
                            step[0] += 1
                            if len(pend) > LAG:
                                attn_v(*pend.pop(0))
                                flush_norms(NORMLAG)
                                pump(PUMP)
                while pend:
                    attn_v(*pend.pop(0))
                flush_norms(0)

            def normalize(h, cq, ps_o):
                """Evacuate ps_o once (ACT/DVE), then denominator handling and
                the scaled copies run on the otherwise-idle GPSIMD engine so
                the exp engines stay dense. The last chunk takes the direct
                low-latency path instead (exp engines are draining by then)."""
                den = rpool.tile([P, NQ], F32, name="den", tag="den")
                rec = rpool.tile([P, NQ], F32, name="rec", tag="rec")
                rec_keep.append(rec)
                if cq == NQ - 1 and h == HPC - 1:
                    if V8:
                        nc.vector.tensor_scalar(
                            den, ps_o[:, :, D], 16.0, None, mybir.AluOpType.mult
                        )
                    else:
                        nc.vector.tensor_copy(den, ps_o[:, :, D])
                    nc.vector.reciprocal(rec, den)
                    for qt in range(NQ):
                        st = cq * NQ + qt
                        dst = o_sb[st][:, h * D : (h + 1) * D]
                        if qt % 2 == 0:
                            nc.scalar.activation(
                                out=dst, in_=ps_o[:, qt, 0:D],
                                func=mybir.ActivationFunctionType.Copy,
                                scale=rec[:, qt : qt + 1],
                            )
                        else:
                            nc.vector.tensor_scalar(
                                dst, ps_o[:, qt, 0:D], rec[:, qt : qt + 1],
                                None, mybir.AluOpType.mult,
                            )
                    return
                po_sb = npool.tile([P, NQ, D + 1], F32, name="po_sb", tag="po_sb")
                if h % 2 == 0:
                    nc.scalar.activation(
                        out=po_sb, in_=ps_o,
                        func=mybir.ActivationFunctionType.Copy,
                    )
                else:
                    nc.vector.tensor_copy(po_sb, ps_o)
                if V8:
                    nc.gpsimd.tensor_scalar(
                        den, po_sb[:, :, D], 16.0, None, mybir.AluOpType.mult
                    )
                else:
                    nc.gpsimd.tensor_copy(den, po_sb[:, :, D])
                nc.vector.reciprocal(rec, den)
                for qt in range(NQ):
                    st = cq * NQ + qt
                    nc.gpsimd.tensor_scalar(
                        o_sb[st][:, h * D : (h + 1) * D],
                        po_sb[:, qt, 0:D],
                        rec[:, qt : qt + 1],
                        None,
                        mybir.AluOpType.mult,
                    )

            def finish(cq):
                """Transpose + output projection for the 4 s-tiles of cq.
                The last chunk runs post-attention: use the freed score psum
                pool for deeper pipelining and the idle ACT/DVE for copies."""
                last = cq == NQ - 1
                for qt in range(NQ):
                    st = cq * NQ + qt
                    oT = otpool.tile([P, 2, P], BF16, name="oT", tag="oT")
                    for ct in range(2):
                        tp = mpool.tile([P, P], BF16, name="tp", tag="m")
                        nc.tensor.transpose(
                            tp, o_sb[st][:, ct * P : (ct + 1) * P], ident
                        )
                        yield
                        nc.vector.tensor_copy(oT[:, ct, :], tp)
                        yield
                    if DBG and st == 0:
                        nc.sync.dma_start(out=dbg_oT.ap(), in_=oT)
                    out_sb = outpool.tile([P, E], BF16, name="out_sb", tag="out_sb")
                    for gc in range(2):
                        if last:
                            po = spool.tile([P, QC], F32, name="sps", tag="sps")
                        else:
                            po = mpool.tile([P, QC], F32, name="po", tag="m")
                        for ct in range(2):
                            nc.tensor.matmul(
                                po,
                                oT[:, ct, :],
                                wo[:, ct, gc * QC : (gc + 1) * QC],
                                start=(ct == 0),
                                stop=(ct == 1),
                            )
                            yield
                        gsl = slice(gc * QC, (gc + 1) * QC)
                        if gc == 0:
                            nc.scalar.activation(
                                out=out_sb[:, gsl], in_=po,
                                func=mybir.ActivationFunctionType.Copy,
                            )
                        else:
                            nc.vector.tensor_copy(out_sb[:, gsl], po)
                        yield
                        if GCDMA or last:
                            nc.sync.dma_start(
                                out=out_d.ap()[st * P : (st + 1) * P, gsl],
                                in_=out_sb[:, gsl],
                            )
                    if not (GCDMA or last):
                        nc.sync.dma_start(
                            out=out_d.ap()[st * P : (st + 1) * P, :], in_=out_sb
                        )

            attn_all()
            while fillers:
                pump(64)

    nc.compile()
    return nc


_NC_CACHE = None


def _get_nc():
    global _NC_CACHE
    if _NC_CACHE is None:
        _NC_CACHE = _build()
    return _NC_CACHE


def _bf16(a):
    return np.ascontiguousarray(a.astype(ml_dtypes.bfloat16))


def _f8(a):
    return np.ascontiguousarray(a.astype(ml_dtypes.float8_e4m3fn))


def make_in_maps(x, Wq, Wk, Wv, Wo):
    in_maps = []
    xTs = [_bf16(x[b].T) for b in range(B)]
    ident = np.eye(P, dtype=ml_dtypes.bfloat16)
    for c in range(NCORES):
        b, hg = c // GPB, c % GPB
        fsl = slice(hg * FPC, (hg + 1) * FPC)
        m = {
            "xT": xTs[b],
            "wqT": _bf16(Wq[fsl, :].T * SM),
            "wkT": _bf16(Wk[fsl, :].T),
            "wvT": _bf16(Wv[fsl, :].T),
            "woT": _bf16(Wo[:, fsl].T),
            "ident": ident,
        }
        if PROJ8 or V8:
            m["x8"] = _f8(x[b].T)
        if PROJ8:
            m["wq8"] = _f8(Wq[fsl, :].T * 16.0)
            m["wk8"] = _f8(Wk[fsl, :].T * 16.0)
        if V8:
            m["wv8"] = _f8(Wv[fsl, :].T * 16.0)
        in_maps.append(m)
    return in_maps


def kernel(x, Wq, bq, Wk, bk, Wv, bv, Wo, bo):
    x = np.asarray(x, dtype=np.float32)
    Wq, Wk, Wv, Wo = (np.asarray(a, dtype=np.float32) for a in (Wq, Wk, Wv, Wo))
    bq, bk, bv, bo = (np.asarray(a, dtype=np.float32) for a in (bq, bk, bv, bo))
    if np.any(bq) or np.any(bk) or np.any(bv):
        raise NotImplementedError("nonzero projection biases not supported")

    nc = _get_nc()
    in_maps = make_in_maps(x, Wq, Wk, Wv, Wo)
    res = run_bass_kernel_spmd(nc, in_maps, core_ids=list(range(NCORES)))
    out = np.empty((B, S, E), dtype=np.float32)
    for b in range(B):
        acc = res.results[b * GPB]["out"].astype(np.float32)
        for hg in range(1, GPB):
            acc = acc + res.results[b * GPB + hg]["out"].astype(np.float32)
        out[b] = acc
    out += bo[None, None, :]
    return out


# revision 7
# speedup vs baseline: 1.0302x; 1.0019x over previous
"""Trainium2 Bass kernel v2: multi-head attention (B=2, S=2048, E=1024, H=16).

Sharding: 8 cores = 2 batches x 4 head-groups; core c handles batch c//4,
heads [4*(c%4), 4*(c%4)+4).

Per-core program (all matmuls bf16, fp32 psum accumulate):
  - q/k projected directly into [d, s] layout (transposed matmuls); v into
    [s, c] layout with a ones column per head.
  - scores^T tiles [128 k, 512 q] on PE (sm_scale folded into Wq host-side);
    exp split across ACT (native Exp) and DVE/GPSIMD (Schraudolph:
    int16(A*s + B) bit-cast to bf16; the approximation's constant factor
    cancels in softmax normalization).
  - attn@v with attn^T chunks [128 k, 128 q] stationary and v [128 k, 65]
    moving -> psum [128 q, 65] (full 128-row contraction; col 64 = softmax
    denominator). reciprocal_approx_fast + per-partition scaled copy
    normalizes into o [s, c] bf16.
  - o transposed via PE identity-matmul, then output projection; partial
    [S, E] written bf16; host sums the 4 head-group partials per batch.
"""

import numpy as np
import ml_dtypes

import concourse.tile as tile
import concourse.mybir as mybir
from concourse import bacc
from concourse.bass_utils import run_bass_kernel_spmd

B, S, E, H, D = 2, 2048, 1024, 16, 64
NCORES = 8
GPB = NCORES // B      # head-group cores per batch = 4
HPC = H // GPB         # heads per core = 4
FPC = HPC * D          # feature cols per core = 256
SM = float(D) ** -0.5  # softmax scale (folded into Wq on host)

F32 = mybir.dt.float32
F8 = mybir.dt.float8e4
BF16 = mybir.dt.bfloat16
I16 = mybir.dt.int16

P = 128
NE = E // P            # 8 e-tiles (contraction chunks)
NST = S // P           # 16 s-tiles (key tiles)
NQ = 4                 # query chunks
QC = S // NQ           # 512
NJ = NST // 2          # 8 kt-pairs per (head, chunk)

import os

# Schraudolph exp in bf16: exp(s) ~= bitcast_bf16(int16(A16*s + B16));
# B16 = 16256 + c with c chosen so E[ln(approx/exp)] = 0 over the mantissa
# ripple -- matches the ACT-exp blocks so softmax block weights stay unbiased
A16 = 128.0 * 1.4426950408889634
B16 = float(os.environ.get("KV2_B16", "16249.5"))

# exp engine pattern per key tile kt (A=ACT native, D=DVE Schraudolph)
_PATS = {
    "AD": ["A", "A", "D", "A", "D", "A", "A", "D",
           "A", "D", "A", "A", "D", "A", "D", "A"],
    "AD8": ["A", "D", "A", "D", "A", "D", "A", "D",
            "A", "D", "A", "D", "A", "D", "A", "D"],
    "AD7": ["A", "D", "A", "D", "A", "D", "A", "A",
            "D", "A", "D", "A", "D", "A", "D", "A"],
    "ALLA": ["A"] * 16,
    "ALLD": ["D"] * 16,
    "AD6": ["A", "D", "A", "A", "D", "A", "D", "A",
            "A", "D", "A", "A", "D", "A", "D", "A"],
    "AD5": ["A", "D", "A", "A", "D", "A", "A", "D",
            "A", "A", "D", "A", "A", "D", "A", "A"],
    "AD4": ["A", "A", "D", "A", "A", "A", "D", "A",
            "A", "A", "D", "A", "A", "A", "D", "A"],
}
EXP_PAT = _PATS[os.environ.get("KV2_PAT", "AD7")]
PUMP = int(os.environ.get("KV2_PUMP", "1"))
SPOOL = int(os.environ.get("KV2_SPOOL", "4"))
MPOOL = int(os.environ.get("KV2_MPOOL", "2"))
GCDMA = os.environ.get("KV2_GCDMA", "1") == "1"
NORMSPREAD = os.environ.get("KV2_NORMSPREAD", "0") == "1"
LAG = int(os.environ.get("KV2_LAG", "12"))
FP8 = os.environ.get("KV2_FP8", "0") == "1"
APOOL = int(os.environ.get("KV2_APOOL", "14"))
XBATCH = os.environ.get("KV2_XBATCH", "0") == "1"
NORMLAG = int(os.environ.get("KV2_NORMLAG", "5"))
WEXP = os.environ.get("KV2_WEXP", "0") == "1"
GPAT = os.environ.get("KV2_GPAT", "ADADADAAADADADAD")
PROJ8 = os.environ.get("KV2_PROJ8", "0") == "1"
V8 = os.environ.get("KV2_V8", "0") == "1"


def _build():
    nc = bacc.Bacc("TRN2", target_bir_lowering=False, debug=False)

    xT_d = nc.dram_tensor("xT", [E, S], BF16, kind="ExternalInput")
    wq_d = nc.dram_tensor("wqT", [E, FPC], BF16, kind="ExternalInput")
    wk_d = nc.dram_tensor("wkT", [E, FPC], BF16, kind="ExternalInput")
    wv_d = nc.dram_tensor("wvT", [E, FPC], BF16, kind="ExternalInput")
    wo_d = nc.dram_tensor("woT", [FPC, E], BF16, kind="ExternalInput")
    id_d = nc.dram_tensor("ident", [P, P], BF16, kind="ExternalInput")
    if PROJ8 or V8:
        x8_d = nc.dram_tensor("x8", [E, S], F8, kind="ExternalInput")
    if PROJ8:
        wq8_d = nc.dram_tensor("wq8", [E, FPC], F8, kind="ExternalInput")
        wk8_d = nc.dram_tensor("wk8", [E, FPC], F8, kind="ExternalInput")
    if V8:
        wv8_d = nc.dram_tensor("wv8", [E, FPC], F8, kind="ExternalInput")
    out_d = nc.dram_tensor("out", [S, E], BF16, kind="ExternalOutput")
    DBG = os.environ.get("KV2_DEBUG", "0") == "1"
    if DBG:
        dbg_o = nc.dram_tensor("dbg_o", [P, FPC], BF16, kind="ExternalOutput")
        dbg_oT = nc.dram_tensor("dbg_oT", [P, 2, P], BF16, kind="ExternalOutput")
        dbg_rec = nc.dram_tensor("dbg_rec", [P, NQ], F32, kind="ExternalOutput")
        dbg_v = nc.dram_tensor("dbg_v", [P, HPC, D + 1], BF16, kind="ExternalOutput")
        dbg_k = nc.dram_tensor("dbg_k", [P, S], BF16, kind="ExternalOutput")
        dbg_q = nc.dram_tensor("dbg_q", [P, S], BF16, kind="ExternalOutput")
        dbg_at = nc.dram_tensor("dbg_at", [P, QC], BF16, kind="ExternalOutput")
        dbg_po = nc.dram_tensor("dbg_po", [P, NQ, D + 1], F32, kind="ExternalOutput")

    with tile.TileContext(nc) as tc:
        with (
            tc.tile_pool(name="wpool", bufs=1) as wpool,
            tc.tile_pool(name="xpool", bufs=1) as xpool,
            tc.tile_pool(name="qkpool", bufs=1) as qkpool,
            tc.tile_pool(name="vpool", bufs=1) as vpool,
            tc.tile_pool(name="apool", bufs=APOOL) as apool,
            tc.tile_pool(name="opool", bufs=1) as opool,
            tc.tile_pool(name="otpool", bufs=3) as otpool,
            tc.tile_pool(name="rpool", bufs=2) as rpool,
            tc.tile_pool(name="npool", bufs=2) as npool,
            tc.tile_pool(name="outpool", bufs=4) as outpool,
            tc.tile_pool(name="spool", bufs=SPOOL, space="PSUM") as spool,
            tc.tile_pool(name="oaccpool", bufs=2, space="PSUM") as oaccpool,
            tc.tile_pool(name="mpool", bufs=MPOOL, space="PSUM") as mpool,
        ):
            # ---- weights / constants -------------------------------------
            wk = wpool.tile([P, NE, FPC], BF16, name="wk")
            wq = wpool.tile([P, NE, FPC], BF16, name="wq")
            wv = wpool.tile([P, NE, FPC], BF16, name="wv")
            wo = wpool.tile([P, 2, E], BF16, name="wo")
            ident = wpool.tile([P, P], BF16, name="ident")

            wk_r = wk_d.ap().rearrange("(t p) f -> p t f", p=P)
            wq_r = wq_d.ap().rearrange("(t p) f -> p t f", p=P)
            if PROJ8:
                # e = pr*256 + sub*128 + p: DoubleRow pairs of e-chunks
                wk8 = wpool.tile([P, NE // 2, 2, FPC], F8, name="wk8")
                wq8 = wpool.tile([P, NE // 2, 2, FPC], F8, name="wq8")
                x8 = xpool.tile([P, NE // 2, 2, S], F8, name="x8")
                wk8_r = wk8_d.ap().rearrange("(r u p) f -> p r u f", p=P, u=2)
                wq8_r = wq8_d.ap().rearrange("(r u p) f -> p r u f", p=P, u=2)
                x8_r = x8_d.ap().rearrange("(r u p) s -> p r u s", p=P, u=2)
                nc.sync.dma_start(out=wk8, in_=wk8_r)
                nc.sync.dma_start(out=wq8, in_=wq8_r)
                nc.sync.dma_start(out=x8[:, :, :, 0:QC], in_=x8_r[:, :, :, 0:QC])
            else:
                # pair-0 halves of Wk/Wq first: they gate the first projections
                nc.sync.dma_start(out=wk[:, :, 0:P], in_=wk_r[:, :, 0:P])
                nc.sync.dma_start(out=wq[:, :, 0:P], in_=wq_r[:, :, 0:P])

            if XBATCH:
                xts_all = xpool.tile([P, NE, S], BF16, name="xt")
                xts = [xts_all[:, et, :] for et in range(NE)]
            else:
                xts = [
                    xpool.tile([P, S], BF16, name=f"xt{et}", tag=f"xt{et}")
                    for et in range(NE)
                ]
            if V8:
                wv8 = wpool.tile([P, NE // 2, 2, FPC], F8, name="wv8")
                x8v = xpool.tile([P, NE // 2, 2, S], F8, name="x8v")
                x8v_r = x8_d.ap().rearrange("(r u p) s -> p r u s", p=P, u=2)
                nc.sync.dma_start(
                    out=wv8, in_=wv8_d.ap().rearrange("(r u p) f -> p r u f", p=P, u=2)
                )
            xT_r = xT_d.ap().rearrange("(t p) s -> p t s", p=P)
            for et in range(NE):
                nc.sync.dma_start(out=xts[et][:, 0:QC], in_=xT_r[:, et, 0:QC])
            if V8:
                nc.sync.dma_start(out=x8v[:, :, :, 0:QC], in_=x8v_r[:, :, :, 0:QC])
            nc.sync.dma_start(
                out=wv, in_=wv_d.ap().rearrange("(t p) f -> p t f", p=P)
            )
            if not PROJ8:
                nc.sync.dma_start(out=wk[:, :, P:FPC], in_=wk_r[:, :, P:FPC])
                nc.sync.dma_start(out=wq[:, :, P:FPC], in_=wq_r[:, :, P:FPC])
            for cq in range(1, NQ):
                csl = slice(cq * QC, (cq + 1) * QC)
                if PROJ8:
                    nc.sync.dma_start(out=x8[:, :, :, csl], in_=x8_r[:, :, :, csl])
                for et in range(NE):
                    nc.sync.dma_start(out=xts[et][:, csl], in_=xT_r[:, et, csl])
                if V8:
                    nc.sync.dma_start(out=x8v[:, :, :, csl], in_=x8v_r[:, :, :, csl])
            nc.sync.dma_start(
                out=wo, in_=wo_d.ap().rearrange("(t p) g -> p t g", p=P)
            )
            nc.sync.dma_start(out=ident, in_=id_d.ap())

            # ---- persistent sbuf tensors ---------------------------------
            QKDT = F8 if FP8 else BF16
            kts = [qkpool.tile([P, S], QKDT, name=f"kt{p}", tag=f"kt{p}") for p in range(2)]
            qts = [qkpool.tile([P, S], QKDT, name=f"qt{p}", tag=f"qt{p}") for p in range(2)]
            if FP8:
                kf8 = [qkpool.tile([P, 2, S], F8, name=f"kf8{p}", tag=f"kf8{p}") for p in range(2)]
                qf8 = [qkpool.tile([P, 2, S], F8, name=f"qf8{p}", tag=f"qf8{p}") for p in range(2)]

            def shuffle_f8(stage, dst, csl):
                """stage [128, S] fp8 -> dst [128, 2, S]: head tiles at base
                partitions {0,64}; contraction d = sub*32 + p."""
                st_r = stage.rearrange("(b u p) s -> b u p s", b=2, u=2)
                dst_r = dst.rearrange("(b u p) t s -> b u p t s", b=2, u=2)
                for sub in range(2):
                    nc.sync.dma_start(
                        out=dst_r[:, 0, :, sub, csl], in_=st_r[:, sub, :, csl]
                    )
            v_sb = [
                vpool.tile([P, HPC, D + 1], BF16, name=f"v{st}", tag=f"v{st}")
                for st in range(NST)
            ]
            # ones columns for softmax denominators (idle Pool engine, early)
            for st in range(NST):
                nc.gpsimd.memset(v_sb[st][:, :, D : D + 1], 1.0)
            o_sb = [
                opool.tile([P, FPC], BF16, name=f"o{st}", tag=f"o{st % 8}", bufs=2)
                for st in range(NST)
            ]

            # ---- filler machinery (PE backfill units) --------------------
            from collections import deque

            fillers = deque()
            ready = set()

            def pump(n):
                for _ in range(n):
                    while fillers:
                        try:
                            next(fillers[0])
                            break
                        except StopIteration:
                            fillers.popleft()
                    else:
                        return

            def pump_until(key):
                # force-drain fillers until the unit producing `key` has been
                # fully EMITTED (emission order defines Tile dependencies)
                while key not in ready:
                    assert fillers, f"no filler can produce {key}"
                    try:
                        next(fillers[0])
                    except StopIteration:
                        fillers.popleft()

            def kq_proj(w_tile, dst, p, cq, copy_eng=None):
                """dst[p][:, cq chunk] = (x @ W_pslice^T)^T  in [d, s] layout."""
                ps = mpool.tile([P, QC], F32, name="ps_kq", tag="m")
                if PROJ8:
                    w8t = wk8 if dst is kts else wq8
                    for pr in range(NE // 2):
                        nc.tensor.matmul(
                            ps,
                            w8t[:, pr, :, p * P : (p + 1) * P],
                            x8[:, pr, :, cq * QC : (cq + 1) * QC],
                            start=(pr == 0),
                            stop=(pr == NE // 2 - 1),
                            perf_mode=mybir.MatmulPerfMode.DoubleRow,
                        )
                        yield
                else:
                    for et in range(NE):
                        nc.tensor.matmul(
                            ps,
                            w_tile[:, et, p * P : (p + 1) * P],
                            xts[et][:, cq * QC : (cq + 1) * QC],
                            start=(et == 0),
                            stop=(et == NE - 1),
                        )
                        yield
                csl = slice(cq * QC, (cq + 1) * QC)
                dslice = dst[p][:, csl]
                if copy_eng == "A":
                    nc.scalar.activation(
                        out=dslice, in_=ps,
                        func=mybir.ActivationFunctionType.Copy,
                    )
                else:
                    nc.vector.tensor_copy(dslice, ps)
                if FP8 and dst in (kts, qts):
                    shuffle_f8(dst[p], (kf8 if dst is kts else qf8)[p], csl)
                yield

            def v_proj(st):
                """v_sb[st][:, h, 0:D] = x s-tile @ Wv^T (all 4 heads).
                V8: fp8 DoubleRow with x16-lifted operands -> v scaled by 256;
                compensated in the softmax denominator scaling."""
                ps = mpool.tile([P, FPC], F32, name="ps_v", tag="m")
                if V8:
                    for pr in range(NE // 2):
                        nc.tensor.matmul(
                            ps,
                            x8v[:, pr, :, st * P : (st + 1) * P],
                            wv8[:, pr, :, :],
                            start=(pr == 0),
                            stop=(pr == NE // 2 - 1),
                            perf_mode=mybir.MatmulPerfMode.DoubleRow,
                        )
                        yield
                else:
                    for et in range(NE):
                        nc.tensor.matmul(
                            ps,
                            xts[et][:, st * P : (st + 1) * P],
                            wv[:, et, :],
                            start=(et == 0),
                            stop=(et == NE - 1),
                        )
                        yield
                nc.scalar.activation(
                    out=v_sb[st][:, :, 0:D],
                    in_=ps.rearrange("p (h d) -> p h d", d=D),
                    func=mybir.ActivationFunctionType.Copy,
                )
                yield

            def run_now(gen):
                for _ in gen:
                    pass

            # ---- upfront: k/q chunk 0 (pair 0), v tiles 0-1; rest filler --
            run_now(kq_proj(wk, kts, 0, 0))
            run_now(kq_proj(wq, qts, 0, 0, copy_eng="A"))
            ready.update({"k0c0", "q0c0"})

            def filler_projs():
                for st in (0, 1, 2, 3):
                    yield from v_proj(st)
                    ready.add(f"v{st}")
                yield from kq_proj(wk, kts, 0, 1)
                ready.add("k0c1")
                for st in (4, 5, 6, 7):
                    yield from v_proj(st)
                    ready.add(f"v{st}")
                yield from kq_proj(wk, kts, 0, 2)
                ready.add("k0c2")
                for st in (8, 9, 10, 11):
                    yield from v_proj(st)
                    ready.add(f"v{st}")
                yield from kq_proj(wk, kts, 0, 3)
                ready.add("k0c3")
                for st in range(12, NST):
                    yield from v_proj(st)
                    ready.add(f"v{st}")
                yield from kq_proj(wq, qts, 1, 0, copy_eng="A")
                ready.add("q1c0")
                for cq in range(NQ):
                    yield from kq_proj(wk, kts, 1, cq)
                    ready.add(f"k1c{cq}")
                for cq in range(1, NQ):
                    yield from kq_proj(wq, qts, 0, cq, copy_eng="A")
                    ready.add(f"q0c{cq}")
                    yield from kq_proj(wq, qts, 1, cq)
                    ready.add(f"q1c{cq}")

            fillers.append(filler_projs())

            # ---- attention core ------------------------------------------
            rec_keep = []
            dbg_attn_keep = []

            def attn_all():
                """All (cq, h) attention as a single software-pipelined
                (cq, h, kt) stream: attn@v lags LAG steps behind the
                score/exp stream, crossing head AND chunk boundaries."""
                ps_os = {}
                attn_ts = {}
                pend = []
                norm_pend = []
                step = [0]

                def flush_norms(min_age):
                    while norm_pend and step[0] - norm_pend[0][0] >= min_age:
                        _, ncq, nh, nps = norm_pend.pop(0)
                        normalize(nh, ncq, nps)
                        if nh == HPC - 1:
                            fillers.append(finish(ncq))

                def attn_v(cq, h, kt):
                    at = attn_ts.pop((cq, h, kt))
                    ps_o = ps_os[(cq, h)]
                    pump_until(f"v{kt}")
                    for qt in range(NQ):
                        # start=True clears has_written for the WHOLE bank:
                        # only the first matmul may carry it; qt>0 at kt==0
                        # rely on cleared bits -> overwrite semantics.
                        nc.tensor.matmul(
                            ps_o[:, qt, :],
                            at[:, qt * P : (qt + 1) * P],
                            v_sb[kt][:, h, :],
                            start=(kt == 0 and qt == 0),
                            stop=(kt == NST - 1 and qt == NQ - 1),
                            skip_group_check=(kt == 0 and qt > 0),
                        )
                    if kt == NST - 1:
                        norm_pend.append((step[0], cq, h, ps_os.pop((cq, h))))

                for cq in range(NQ):
                    for h in range(HPC):
                        p, sub = h // 2, h % 2
                        lo = sub * D
                        csl = slice(cq * QC, (cq + 1) * QC)
                        ps_os[(cq, h)] = oaccpool.tile(
                            [P, NQ, D + 1], F32, name="ps_o", tag="oacc"
                        )
                        pump_until(f"q{p}c{cq}")
                        sps2 = None
                        for kt in range(NST):
                            pump_until(f"k{p}c{kt // NQ}")
                            if WEXP:
                                if kt % 2 == 0:
                                    sps2 = spool.tile(
                                        [P, 2, QC], F32, name="sps", tag="sps"
                                    )
                                tgt = sps2[:, kt % 2, :]
                            else:
                                tgt = spool.tile([P, QC], F32, name="sps", tag="sps")
                            if FP8:
                                nc.tensor.matmul(
                                    tgt,
                                    kf8[p][lo : lo + 32, :, kt * P : (kt + 1) * P],
                                    qf8[p][lo : lo + 32, :, csl],
                                    start=True,
                                    stop=True,
                                    perf_mode=mybir.MatmulPerfMode.DoubleRow,
                                )
                            else:
                                nc.tensor.matmul(
                                    tgt,
                                    kts[p][lo : lo + D, kt * P : (kt + 1) * P],
                                    qts[p][lo : lo + D, csl],
                                    start=True,
                                    stop=True,
                                )
                            pump(PUMP)
                            if WEXP:
                                if kt % 2 == 1:
                                    attn_t = apool.tile(
                                        [P, 2, QC], BF16, name="attn", tag="attn"
                                    )
                                    attn_ts[(cq, h, kt - 1)] = attn_t[:, 0, :]
                                    attn_ts[(cq, h, kt)] = attn_t[:, 1, :]
                                    g = (h % 2) * 8 + kt // 2
                                    if GPAT[g] == "A":
                                        nc.scalar.activation(
                                            out=attn_t,
                                            in_=sps2,
                                            func=mybir.ActivationFunctionType.Exp,
                                        )
                                    else:
                                        nc.vector.tensor_scalar(
                                            attn_t.bitcast(I16),
                                            sps2,
                                            A16,
                                            B16,
                                            mybir.AluOpType.mult,
                                            mybir.AluOpType.add,
                                        )
                            else:
                                attn_t = apool.tile(
                                    [P, QC], BF16, name="attn", tag="attn"
                                )
                                attn_ts[(cq, h, kt)] = attn_t
                                esc = SM / 256.0 if PROJ8 else 1.0
                                if EXP_PAT[kt] == "A":
                                    nc.scalar.activation(
                                        out=attn_t,
                                        in_=tgt,
                                        func=mybir.ActivationFunctionType.Exp,
                                        scale=esc,
                                    )
                                else:
                                    nc.vector.tensor_scalar(
                                        attn_t.bitcast(I16),
                                        tgt,
                                        A16 * esc,
                                        B16,
                                        mybir.AluOpType.mult,
                                        mybir.AluOpType.add,
                                    )
                            pend.append((cq, h, kt))
                            step[0] += 1
                            if len(pend) > LAG:
                                attn_v(*pend.pop(0))
                                flush_norms(NORMLAG)
                                pump(PUMP)
                while pend:
                    attn_v(*pend.pop(0))
                flush_norms(0)

            def normalize(h, cq, ps_o):
                """Evacuate ps_o once (ACT/DVE), then denominator handling and
                the scaled copies run on the otherwise-idle GPSIMD engine so
                the exp engines stay dense. The last chunk takes the direct
                low-latency path instead (exp engines are draining by then)."""
                den = rpool.tile([P, NQ], F32, name="den", tag="den")
                rec = rpool.tile([P, NQ], F32, name="rec", tag="rec")
                rec_keep.append(rec)
                if cq == NQ - 1 and h == HPC - 1:
                    if V8:
                        nc.vector.tensor_scalar(
                            den, ps_o[:, :, D], 16.0, None, mybir.AluOpType.mult
                        )
                    else:
                        nc.vector.tensor_copy(den, ps_o[:, :, D])
                    nc.vector.reciprocal(rec, den)
                    for qt in range(NQ):
                        st = cq * NQ + qt
                        dst = o_sb[st][:, h * D : (h + 1) * D]
                        if qt % 2 == 0:
                            nc.scalar.activation(
                                out=dst, in_=ps_o[:, qt, 0:D],
                                func=mybir.ActivationFunctionType.Copy,
                                scale=rec[:, qt : qt + 1],
                            )
                        else:
                            nc.vector.tensor_scalar(
                                dst, ps_o[:, qt, 0:D], rec[:, qt : qt + 1],
                                None, mybir.AluOpType.mult,
                            )
                    return
                po_sb = npool.tile([P, NQ, D + 1], F32, name="po_sb", tag="po_sb")
                if h % 2 == 0:
                    nc.scalar.activation(
                        out=po_sb, in_=ps_o,
                        func=mybir.ActivationFunctionType.Copy,
                    )
                else:
                    nc.vector.tensor_copy(po_sb, ps_o)
                if V8:
                    nc.gpsimd.tensor_scalar(
                        den, po_sb[:, :, D], 16.0, None, mybir.AluOpType.mult
                    )
                else:
                    nc.gpsimd.tensor_copy(den, po_sb[:, :, D])
                nc.vector.reciprocal(rec, den)
                for qt in range(NQ):
                    st = cq * NQ + qt
                    nc.gpsimd.tensor_scalar(
                        o_sb[st][:, h * D : (h + 1) * D],
                        po_sb[:, qt, 0:D],
                        rec[:, qt : qt + 1],
                        None,
                        mybir.AluOpType.mult,
                    )

            def finish(cq):
                """Transpose + output projection for the 4 s-tiles of cq.
                The last chunk runs post-attention: use the freed score psum
                pool for deeper pipelining and the idle ACT/DVE for copies."""
                last = cq == NQ - 1
                for qt in range(NQ):
                    st = cq * NQ + qt
                    oT = otpool.tile([P, 2, P], BF16, name="oT", tag="oT")
                    for ct in range(2):
                        tp = mpool.tile([P, P], BF16, name="tp", tag="m")
                        nc.tensor.transpose(
                            tp, o_sb[st][:, ct * P : (ct + 1) * P], ident
                        )
                        yield
                        nc.vector.tensor_copy(oT[:, ct, :], tp)
                        yield
                    if DBG and st == 0:
                        nc.sync.dma_start(out=dbg_oT.ap(), in_=oT)
                    out_sb = outpool.tile([P, E], BF16, name="out_sb", tag="out_sb")
                    for gc in range(2):
                        if last:
                            po = spool.tile([P, QC], F32, name="sps", tag="sps")
                        else:
                            po = mpool.tile([P, QC], F32, name="po", tag="m")
                        for ct in range(2):
                            nc.tensor.matmul(
                                po,
                                oT[:, ct, :],
                                wo[:, ct, gc * QC : (gc + 1) * QC],
                                start=(ct == 0),
                                stop=(ct == 1),
                            )
                            yield
                        gsl = slice(gc * QC, (gc + 1) * QC)
                        if gc == 0:
                            nc.scalar.activation(
                                out=out_sb[:, gsl], in_=po,
                                func=mybir.ActivationFunctionType.Copy,
                            )
                        else:
                            nc.vector.tensor_copy(out_sb[:, gsl], po)
                        yield
                        if GCDMA or last:
                            nc.sync.dma_start(
                                out=out_d.ap()[st * P : (st + 1) * P, gsl],
                                in_=out_sb[:, gsl],
                            )
                    if not (GCDMA or last):
                        nc.sync.dma_start(
                            out=out_d.ap()[st * P : (st + 1) * P, :], in_=out_sb
                        )

            attn_all()
            while fillers:
                pump(64)

    nc.compile()
    return nc


_NC_CACHE = None


def _get_nc():
    global _NC_CACHE
    if _NC_CACHE is None:
        _NC_CACHE = _build()
    return _NC_CACHE


def _bf16(a):
    return np.ascontiguousarray(a.astype(ml_dtypes.bfloat16))


def _f8(a):
    return np.ascontiguousarray(a.astype(ml_dtypes.float8_e4m3fn))


def make_in_maps(x, Wq, Wk, Wv, Wo):
    in_maps = []
    xTs = [_bf16(x[b].T) for b in range(B)]
    ident = np.eye(P, dtype=ml_dtypes.bfloat16)
    for c in range(NCORES):
        b, hg = c // GPB, c % GPB
        fsl = slice(hg * FPC, (hg + 1) * FPC)
        m = {
            "xT": xTs[b],
            "wqT": _bf16(Wq[fsl, :].T * SM),
            "wkT": _bf16(Wk[fsl, :].T),
            "wvT": _bf16(Wv[fsl, :].T),
            "woT": _bf16(Wo[:, fsl].T),
            "ident": ident,
        }
        if PROJ8 or V8:
            m["x8"] = _f8(x[b].T)
        if PROJ8:
            m["wq8"] = _f8(Wq[fsl, :].T * 16.0)
            m["wk8"] = _f8(Wk[fsl, :].T * 16.0)
        if V8:
            m["wv8"] = _f8(Wv[fsl, :].T * 16.0)
        in_maps.append(m)
    return in_maps


def kernel(x, Wq, bq, Wk, bk, Wv, bv, Wo, bo):
    x = np.asarray(x, dtype=np.float32)
    Wq, Wk, Wv, Wo = (np.asarray(a, dtype=np.float32) for a in (Wq, Wk, Wv, Wo))
    bq, bk, bv, bo = (np.asarray(a, dtype=np.float32) for a in (bq, bk, bv, bo))
    if np.any(bq) or np.any(bk) or np.any(bv):
        raise NotImplementedError("nonzero projection biases not supported")

    nc = _get_nc()
    in_maps = make_in_maps(x, Wq, Wk, Wv, Wo)
    res = run_bass_kernel_spmd(nc, in_maps, core_ids=list(range(NCORES)))
    out = np.empty((B, S, E), dtype=np.float32)
    for b in range(B):
        acc = res.results[b * GPB]["out"].astype(np.float32)
        for hg in range(1, GPB):
            acc = acc + res.results[b * GPB + hg]["out"].astype(np.float32)
        out[b] = acc
    out += bo[None, None, :]
    return out
